# revision 13
# baseline (speedup 1.0000x reference)
"""Causal self-attention (B=4, L=2048, D=1536, H=24, RoPE) on 8 trn2 NeuronCores.

Sharding: hybrid batch x head-group tensor parallel. Core c handles batch
c//2 with head group c%2 (12 of 24 heads). Per-core work: QKV projection for
its heads, RoPE, causal attention, and a partial output projection over its
768 input features. Partial outputs are summed pairwise (cores 2b, 2b+1 share
batch b) with token-chunked 2-rank ReduceScatters that overlap the output
projection; the host only reassembles chunks.

Device layout choices (all matmuls are lhsT.T @ rhs with K on partitions):
- Phase 1b computes V first (natural token x feature layout, serves as PV
  lhsT). Phase 1a computes qkT (features x tokens) with fused RoPE, in
  per-head-pair m order so attention can start early.
- Softmax runs on S^T: exp on the scalar engine (scale folded in; no max
  subtraction -- scores are O(1), exp cannot overflow), causal masking by
  binary-mask multiply after exp (merged across block pairs), denominator
  free as row 64 of the P@V matmul via a ones-column in V (M=65).
- Softmax epilogue is decoupled from the PE critical path: PV PSUM is
  evacuated by one DVE copy, then reciprocal_approx_fast + gpsimd broadcast
  + DVE multiply run behind the next block's matmuls (psO double-buffered).
- fp32r everywhere (full PE rate at N>=256), fp32 accumulation in PSUM.
"""

import os
import sys

sys.path.insert(0, "/opt/trn_rl_repo")

import numpy as np

import concourse.bass as bass
import concourse.mybir as mybir
import concourse.tile as tile
from concourse import bacc
from concourse.bass_utils import run_bass_kernel_spmd

P = 128
B, L, D = 4, 2048, 1536
H, DH = 24, 64
HL = 12            # heads per core
NPAIR = 6          # head pairs per core
KC = D // P        # 12 contraction chunks for D
QF = HL * DH       # 768 q (or k) features per core
NT = L // 512      # 4 token tiles of 512
TC = L // P        # 16 token chunks of 128
NG = 4             # ReduceScatter token groups
GT = L // NG       # tokens per group (512)
ROPE_BASE = 10000.0

f32 = mybir.dt.float32
f32r = mybir.dt.float32r
f16 = mybir.dt.float16

_CACHE = {}
LAST_RESULT = None


def _build_nc():
    nc = bacc.Bacc(
        "TRN2",
        target_bir_lowering=False,
        debug=False,
        enable_asserts=True,
        num_devices=8,
    )

    xT = nc.dram_tensor("xT", [D, L], f16, kind="ExternalInput")
    wqkT = nc.dram_tensor("wqkT", [D, 2 * QF], f16, kind="ExternalInput")
    wvT = nc.dram_tensor("wvT", [D, QF], f16, kind="ExternalInput")
    woutT = nc.dram_tensor("woutT", [QF, D], f16, kind="ExternalInput")
    cosT = nc.dram_tensor("cosT", [P, L], f16, kind="ExternalInput")
    sinT = nc.dram_tensor("sinT", [P, L], f16, kind="ExternalInput")
    rotT = nc.dram_tensor("rotT", [P, P], f16, kind="ExternalInput")
    maskT = nc.dram_tensor("maskT", [P, 4, 512], f16, kind="ExternalInput")
    ones_d = nc.dram_tensor("ones", [P, TC], f16, kind="ExternalInput")
    out_ext = nc.dram_tensor("out", [L // 2, D], f32, kind="ExternalOutput")

    Exp = mybir.ActivationFunctionType.Exp

    # phase-1a m-chunk order: q/k rows of head pair hp adjacent, so the
    # attention for hp can begin as soon as its projection lands in DRAM.
    m_order = []
    for hp in range(NPAIR):
        m_order += [hp, NPAIR + hp]

    with tile.TileContext(nc) as tc:
        with tc.tile_pool(name="dram", bufs=1, space="DRAM") as dram:
            qkT_dram = dram.tile([2 * QF, L], f16)
            v_dram = dram.tile([L, QF], f16)
            partial = [dram.tile([GT, D], f32, name=f"part{g}") for g in range(NG)]
            rs_out = [
                dram.tile([GT // 2, D], f32, name=f"rs{g}") for g in range(NG)
            ]

            # ---------------- Phase 1: projections + RoPE ----------------
            with (
                tc.tile_pool(name="p1x", bufs=1) as p1x,
                tc.tile_pool(name="p1c", bufs=1) as p1c,
                tc.tile_pool(name="p1w", bufs=2) as p1w,
                tc.tile_pool(name="p1s", bufs=3) as p1s,
                tc.tile_pool(name="p1o", bufs=3) as p1o,
                tc.tile_pool(name="p1t", bufs=3) as p1t,
                tc.tile_pool(name="p1v", bufs=3) as p1v,
                tc.tile_pool(name="ps1", bufs=6, space="PSUM") as ps1,
                tc.tile_pool(name="psr", bufs=2, space="PSUM") as psr,
            ):
                xT_r = xT.rearrange("(kc p) t -> p kc t", p=P)
                xT_sb = p1x.tile([P, KC, L], f16)
                for tch in range(4):
                    tsl = slice(tch * 512, (tch + 1) * 512)
                    nc.sync.dma_start(xT_sb[:, :, tsl], xT_r[:, :, tsl])
                rot_sb = p1c.tile([P, P], f16, tag="rot")
                nc.sync.dma_start(rot_sb[:], rotT[:])
                cos_sb = p1c.tile([P, L], f16, tag="cos")
                nc.sync.dma_start(cos_sb[:], cosT[:])
                sin_sb = p1c.tile([P, L], f16, tag="sin")
                nc.sync.dma_start(sin_sb[:], sinT[:])
                wv_sb = p1c.tile([P, KC, QF], f16, tag="wv")
                nc.sync.dma_start(wv_sb[:], wvT.rearrange("(kc p) f -> p kc f", p=P))

                wqkT_r = wqkT.rearrange("(kc p) f -> p kc f", p=P)

                # 1b: v = x @ wv^T in natural (token, feature) layout.
                # k-major inner keeps the stationary operand resident 2 MMs.
                for mt in range(TC):
                    csl = slice(mt * P, (mt + 1) * P)
                    psv = [
                        ps1.tile([P, 512], f32, tag="ps1", name=f"v{mt}_{h}")
                        for h in range(2)
                    ]
                    for k in range(KC):
                        for half in range(2):
                            fsl = slice(half * 384, (half + 1) * 384)
                            nc.tensor.matmul(
                                psv[half][:, :384],
                                xT_sb[:, k, csl],
                                wv_sb[:, k, fsl],
                                start=(k == 0),
                                stop=(k == KC - 1),
                            )
                    for half in range(2):
                        fsl = slice(half * 384, (half + 1) * 384)
                        vo = p1v.tile([P, 384], f16)
                        nc.vector.tensor_copy(vo[:], psv[half][:, :384])
                        nc.sync.dma_start(v_dram[csl, fsl], vo[:])

                # 1a: qkT = wqk @ x^T with fused RoPE; k-major inner reuses
                # the stationary operand across the 4 token tiles.
                for m in m_order:
                    w_t = p1w.tile([P, KC, P], f16)
                    nc.sync.dma_start(w_t[:], wqkT_r[:, :, m * P : (m + 1) * P])
                    psn = [
                        ps1.tile([P, 512], f32, tag="ps1", name=f"qk{m}_{n}")
                        for n in range(NT)
                    ]
                    for k in range(KC):
                        for n in range(NT):
                            nc.tensor.matmul(
                                psn[n][:],
                                w_t[:, k, :],
                                xT_sb[:, k, n * 512 : (n + 1) * 512],
                                start=(k == 0),
                                stop=(k == KC - 1),
                            )
                    for n in range(NT):
                        tsl = slice(n * 512, (n + 1) * 512)
                        stage = p1s.tile([P, 512], f16)
                        nc.scalar.copy(stage[:], psn[n][:])
                        prot = psr.tile([P, 512], f32)
                        nc.tensor.matmul(
                            prot[:], rot_sb[:], stage[:], start=True, stop=True
                        )
                        qko = p1o.tile([P, 512], f16)
                        nc.vector.tensor_mul(qko[:], stage[:], cos_sb[:, tsl])
                        tmp = p1t.tile([P, 512], f16)
                        nc.vector.tensor_mul(tmp[:], prot[:], sin_sb[:, tsl])
                        nc.vector.tensor_add(qko[:], qko[:], tmp[:])
                        nc.sync.dma_start(qkT_dram[m * P : (m + 1) * P, tsl], qko[:])

            # ---------------- Phase 2 + 3 ----------------
            with (
                tc.tile_pool(name="ao", bufs=1) as aop,
                tc.tile_pool(name="p3w", bufs=1) as p3w,
            ):
                attn = aop.tile([P, NPAIR, L], f16)
                wo_sb = p3w.tile([P, NPAIR, D], f16)
                nc.sync.dma_start(
                    wo_sb[:], woutT.rearrange("(kc p) e -> p kc e", p=P)
                )

                with (
                    tc.tile_pool(name="p2k", bufs=2) as p2k,
                    tc.tile_pool(name="p2v", bufs=2) as p2v,
                    tc.tile_pool(name="p2q", bufs=2) as p2q,
                    tc.tile_pool(name="p2p", bufs=4) as p2p,
                    tc.tile_pool(name="p2m", bufs=1) as p2m,
                    tc.tile_pool(name="p2s", bufs=4) as p2s,
                    tc.tile_pool(name="p2r", bufs=4) as p2r,
                    tc.tile_pool(name="p2b", bufs=4) as p2b,
                    tc.tile_pool(name="p2t", bufs=3) as p2t,
                    tc.tile_pool(name="psS", bufs=2, space="PSUM") as psS,
                    tc.tile_pool(name="psO", bufs=4, space="PSUM") as psO,
                ):
                    mask_sb = p2m.tile([P, 4, 512], f16)
                    nc.sync.dma_start(mask_sb[:], maskT[:])
                    v_r = v_dram.rearrange("(kc p) f -> p kc f", p=P)

                    for hp in range(NPAIR):
                        kT = p2k.tile([P, L], f16)
                        nc.sync.dma_start(
                            kT[:], qkT_dram[QF + hp * P : QF + (hp + 1) * P, :]
                        )
                        qT = p2q.tile([P, L], f16)
                        nc.sync.dma_start(
                            qT[:], qkT_dram[hp * P : (hp + 1) * P, :]
                        )
                        v1 = p2v.tile([P, TC, 132], f16)
                        nc.sync.dma_start(
                            v1[:, :, 0:64], v_r[:, :, hp * P : hp * P + 64]
                        )
                        nc.sync.dma_start(
                            v1[:, :, 66:130],
                            v_r[:, :, hp * P + 64 : (hp + 1) * P],
                        )
                        nc.sync.dma_start(v1[:, :, 64:65], ones_d[:, :, None])
                        nc.sync.dma_start(v1[:, :, 130:131], ones_d[:, :, None])

                        for i in range(NT):  # q blocks of 512
                            qsl = slice(i * 512, (i + 1) * 512)
                            pso = [
                                psO.tile([P, 512], f32, tag="pso", name=f"pso{hh}")
                                for hh in range(2)
                            ]
                            njb = 4 * i + 4  # causal k blocks of 128
                            for jp in range(njb // 2):
                                g0 = 2 * jp - 4 * i
                                for hh in range(2):
                                    off = hh * 64
                                    pss = psS.tile([P, 2, 512], f32)
                                    for jj in range(2):
                                        j = 2 * jp + jj
                                        nc.tensor.matmul(
                                            pss[:, jj, :],
                                            kT[off : off + 64, j * P : (j + 1) * P],
                                            qT[off : off + 64, qsl],
                                            start=True,
                                            stop=True,
                                        )
                                    pt = p2p.tile([P, 2, 512], f16)
                                    nc.scalar.activation(
                                        pt.rearrange("p a b -> p (a b)"),
                                        pss.rearrange("p a b -> p (a b)"),
                                        Exp,
                                        scale=0.125,
                                    )
                                    if g0 >= 0:
                                        nc.vector.tensor_mul(
                                            pt.rearrange("p a b -> p (a b)"),
                                            pt.rearrange("p a b -> p (a b)"),
                                            mask_sb[:, g0 : g0 + 2, :].rearrange(
                                                "p a b -> p (a b)"
                                            ),
                                        )
                                    for jj in range(2):
                                        j = 2 * jp + jj
                                        nc.tensor.matmul(
                                            pso[hh][0:65, :],
                                            v1[:, j, 66 * hh : 66 * hh + 65],
                                            pt[:, jj, :],
                                            start=(j == 0),
                                            stop=(j == njb - 1),
                                        )
                            for hh in range(2):
                                off = hh * 64
                                rc = p2r.tile([1, 512], f32)
                                nc.vector.reciprocal(rc[:], pso[hh][64:65, :])
                                rbc = p2b.tile([64, 512], f32)
                                nc.gpsimd.partition_broadcast(rbc[:], rc[:])
                                tmp = p2t.tile([64, 512], f16)
                                nc.vector.tensor_mul(
                                    tmp[:], pso[hh][0:64, :], rbc[:]
                                )
                                nc.sync.dma_start(
                                    attn[off : off + 64, hp, qsl], tmp[:]
                                )

                # ---------------- Phase 3: output projection + chunked RS ----
                with (
                    tc.tile_pool(name="p3o", bufs=3) as p3o,
                    tc.tile_pool(name="ps3", bufs=6, space="PSUM") as ps3,
                ):
                    for g in range(NG):
                        for mt in range(4 * g, 4 * g + 4):
                            msl = slice(mt * P, (mt + 1) * P)
                            lsl = slice((mt - 4 * g) * P, (mt - 4 * g + 1) * P)
                            ps = [
                                ps3.tile([P, 512], f32, tag="ps3", name=f"o{mt}_{n}")
                                for n in range(3)
                            ]
                            for k in range(NPAIR):
                                for nt3 in range(3):
                                    esl = slice(nt3 * 512, (nt3 + 1) * 512)
                                    nc.tensor.matmul(
                                        ps[nt3][:],
                                        attn[:, k, msl],
                                        wo_sb[:, k, esl],
                                        start=(k == 0),
                                        stop=(k == NPAIR - 1),
                                    )
                            for nt3 in range(3):
                                esl = slice(nt3 * 512, (nt3 + 1) * 512)
                                ob = p3o.tile([P, 512], f32)
                                nc.vector.tensor_copy(ob[:], ps[nt3][:])
                                nc.sync.dma_start(partial[g][lsl, esl], ob[:])
                        nc.gpsimd.collective_compute(
                            "ReduceScatter",
                            mybir.AluOpType.add,
                            replica_groups=[[0, 1], [2, 3], [4, 5], [6, 7]],
                            ins=[partial[g].opt()],
                            outs=[rs_out[g].opt()],
                        )
                        nc.sync.dma_start(
                            out_ext[g * (GT // 2) : (g + 1) * (GT // 2), :],
                            rs_out[g][:],
                        )

    nc.compile()
    return nc


def _rope_tables(pos_offset):
    inv_freq = 1.0 / (ROPE_BASE ** (np.arange(0, DH, 2, dtype=np.float32) / DH))
    t = np.arange(L, dtype=np.float32) + np.float32(pos_offset)
    freqs = np.outer(t, inv_freq)                      # (L, 32)
    emb = np.concatenate([freqs, freqs], axis=-1)      # (L, 64)
    cosT = np.cos(emb).T.astype(np.float16)            # (64, L)
    sinT = np.sin(emb).T.astype(np.float16)
    cos2 = np.concatenate([cosT, cosT], axis=0)        # (128, L)
    sin2 = np.concatenate([sinT, sinT], axis=0)
    return np.ascontiguousarray(cos2), np.ascontiguousarray(sin2)


def _rot_matrix():
    R = np.zeros((DH, DH), dtype=np.float32)
    R[:32, 32:] = -np.eye(32, dtype=np.float32)
    R[32:, :32] = np.eye(32, dtype=np.float32)
    R2 = np.zeros((P, P), dtype=np.float32)
    R2[:64, :64] = R
    R2[64:, 64:] = R
    return np.ascontiguousarray(R2.T).astype(np.float16)


def _masks():
    m = np.zeros((4, P, 512), dtype=np.float32)
    kr = np.arange(P)[:, None]
    c = np.arange(512)[None, :]
    for g in range(4):
        m[g] = (P * g + kr <= c).astype(np.float32)
    return np.ascontiguousarray(np.transpose(m, (1, 0, 2))).astype(np.float16)


def _make_in_maps(x, w_qkv, w_out, pos_offset):
    x = np.asarray(x, dtype=np.float32)
    w_qkv = np.asarray(w_qkv, dtype=np.float32)
    w_out = np.asarray(w_out, dtype=np.float32)

    cos2, sin2 = _rope_tables(int(pos_offset))
    rotT = _rot_matrix()
    maskT = _masks()

    in_maps = []
    for c in range(8):
        b, g = c // 2, c % 2
        rows_q = slice(g * QF, (g + 1) * QF)
        rows_k = slice(D + g * QF, D + (g + 1) * QF)
        rows_v = slice(2 * D + g * QF, 2 * D + (g + 1) * QF)
        wqkT = np.ascontiguousarray(
            np.concatenate([w_qkv[rows_q], w_qkv[rows_k]], axis=0).T
        ).astype(np.float16)  # (1536, 1536)
        wvT = np.ascontiguousarray(w_qkv[rows_v].T).astype(np.float16)   # (1536, 768)
        woutT = np.ascontiguousarray(
            w_out[:, g * QF : (g + 1) * QF].T
        ).astype(np.float16)  # (768, 1536)
        xT = np.ascontiguousarray(x[b].T).astype(np.float16)     # (1536, 2048)
        in_maps.append(
            {
                "xT": xT,
                "wqkT": wqkT,
                "wvT": wvT,
                "woutT": woutT,
                "cosT": cos2,
                "sinT": sin2,
                "rotT": rotT,
                "maskT": maskT,
                "ones": np.ones((P, TC), dtype=np.float16),
            }
        )
    return in_maps


def _assemble(results):
    out = np.empty((B, L, D), dtype=np.float32)
    hg = GT // 2  # 256 rows per chunk per core
    for b in range(B):
        lo = results[2 * b]["out"]
        hi = results[2 * b + 1]["out"]
        for g in range(NG):
            out[b, g * GT : g * GT + hg] = lo[g * hg : (g + 1) * hg].astype(
                np.float32
            )
            out[b, g * GT + hg : (g + 1) * GT] = hi[
                g * hg : (g + 1) * hg
            ].astype(np.float32)
    return out


def kernel(x, w_qkv, w_out, pos_offset):
    global LAST_RESULT
    if "nc" not in _CACHE:
        _CACHE["nc"] = _build_nc()
    nc = _CACHE["nc"]
    in_maps = _make_in_maps(x, w_qkv, w_out, pos_offset)
    res = run_bass_kernel_spmd(nc, in_maps, list(range(8)))
    LAST_RESULT = res
    return _assemble(res.results)
